# revision 7
# baseline (speedup 1.0000x reference)
"""Trainium2 Bass kernel for nn_MultiHeadAttention (B=2, E=1024, S=2048, H=16).

Sharding: 8 cores = 2 batches x 4 head-groups (4 heads = 256 channels each).
Each core computes its head-group's QKV projections, attention, and a partial
output projection over its 256 channels; the host sums the 4 partials per
batch and adds the host-folded constant (Wo @ bv + bo).

Schedule: the attention phase runs at ACT (exp) line rate; everything
else hides under it.
- Scores for the head pair (j0, j1) are emitted as two row-tiled matmuls
  (tile_position (0,0)/(64,0)) into one [128,1024] PSUM tile, so the pair
  runs concurrently on the PE and LDWEIGHTS ping-pongs into the background
  weight slot.
- One exp (ACT) per tile, one mask multiply (DVE) with a stride-0
  j-broadcast of the mask slice.
- attn@V and the softmax normalization are emitted through a deferred
  queue (depth DELAY) to decouple them from the scores->exp chain.
- The output projection is chopped into [128out, 512q] chunks that are
  pushed into the NEXT q-block's deferred stream, so only the last
  q-block's out-proj is exposed as a tail; its DMAs go out on the idle
  sync engine's HWDGE ring.
- PSUM plan (8 banks): scores 2x[128,1024] (4) + attn accumulators
  2x[128,512] (2) + proj/out-proj chunks 2x[128,512] (2).

Numerics: bf16 matmul inputs with fp32 PSUM accumulation. Softmax without
max-subtraction; the additive mask becomes a multiplicative exp(mask)
(host-precomputed, bf16). vT carries 64 ones-columns per head, so the
attn@V matmul replicates the softmax denominator across PSUM rows 64..127
and the normalization is a partition-aligned copy -> fast-reciprocal ->
multiply (no PE broadcast).
"""

import os
from contextlib import ExitStack

import numpy as np
import ml_dtypes

import concourse.bass as bass
import concourse.tile as tile
from concourse import bacc, mybir
from concourse import bass_utils

BF16 = mybir.dt.bfloat16
F32 = mybir.dt.float32
Exp = mybir.ActivationFunctionType.Exp

B, E, S, H = 2, 1024, 2048, 16
DH = E // H                      # 64
NCORES = 8
GROUPS = 4                       # head groups (cores per batch)
HPC = H // GROUPS                # 4 heads per core
CH = HPC * DH                    # 256 channels per core
A = CH // 128                    # 2 partition chunks of channels
KE = E // 128                    # 8 contraction chunks over E
QB = S // 512                    # 4 q-blocks of 512
QP2 = S // 1024                  # 2 q-block-pairs of 1024
KC = S // 128                    # 16 k-chunks of 128
KT = KC // 2                     # 8 k-chunk pairs (256 rows)
OC = E // 128                    # 8 output-channel chunks
U = 16                           # 128-row k-slices per q-block


def _emit(tc, nc, d, phases="ABC", blevel=4):
    ctx = tc._emit_ctx

    const = ctx.enter_context(tc.tile_pool(name="const", bufs=1))
    xs_pool = ctx.enter_context(tc.tile_pool(name="xs", bufs=6))
    em_pool = ctx.enter_context(tc.tile_pool(name="em", bufs=12))
    w_pool = ctx.enter_context(tc.tile_pool(name="wx", bufs=4))
    sm_pool = ctx.enter_context(tc.tile_pool(name="sm", bufs=3))
    out_pool = ctx.enter_context(tc.tile_pool(name="outp", bufs=3))
    sc_pool = ctx.enter_context(tc.tile_pool(name="sc", bufs=2, space="PSUM"))
    at_pool = ctx.enter_context(tc.tile_pool(name="at", bufs=2, space="PSUM"))
    pr_pool = ctx.enter_context(tc.tile_pool(name="pr", bufs=2, space="PSUM"))

    if "Z" in phases:
        zt = const.tile([128, 64], F32, name="zt")
        nc.vector.memset(zt[:], 0.0)
        return
    # ---- resident constants ----
    wq_sb = const.tile([128, KE, CH], BF16)
    nc.sync.dma_start(wq_sb[:], d["wqT"])
    wk_sb = const.tile([128, KE, CH], BF16)
    nc.sync.dma_start(wk_sb[:], d["wkT"])
    wv_sb = const.tile([128, KE, CH], BF16)
    nc.sync.dma_start(wv_sb[:], d["wvT"])
    wo_sb = const.tile([128, A, E], BF16)
    nc.sync.dma_start(wo_sb[:], d["woT"])
    bq_sb = const.tile([128, A], F32)
    nc.sync.dma_start(bq_sb[:], d["bq"])
    bk_sb = const.tile([128, A], F32)
    nc.sync.dma_start(bk_sb[:], d["bk"])

    qp_sb = const.tile([128, A, S], BF16)
    kp_sb = const.tile([128, A, S], BF16)
    vT_sb = const.tile([128, KC, HPC * 2 * DH], BF16)
    attn_sb = const.tile([128, A, S], BF16)

    # 64 ones-columns per head: the attn@V matmul then replicates the
    # softmax denominator across PSUM rows 64..127, so the normalization
    # needs no PE broadcast (copy -> reciprocal -> mul, partition-aligned)
    for h in range(HPC):
        nc.vector.memset(vT_sb[:, :, 128 * h + 64 : 128 * (h + 1)], 1.0)

    # xv resident (single big contiguous DMA; overlaps early compute)
    xv_sb = const.tile([128, KE, S], BF16)
    nc.sync.dma_start(xv_sb[:], d["xv"])

    # xq second half resident for the interleaved q-projection
    xq1_sb = const.tile([128, KE, 1024], BF16)
    for ke in range(KE):
        nc.sync.dma_start(xq1_sb[:, ke, :], d["xq"][1, ke])

    # timing-variant support: init tensors a skipped phase would have produced
    if "A" not in phases and "B" in phases:
        nc.vector.memset(qp_sb[:], 0.0)
        nc.vector.memset(kp_sb[:], 0.0)
        nc.vector.memset(vT_sb[:], 1.0)
    if ("B" not in phases or blevel < 4) and "C" in phases:
        nc.vector.memset(attn_sb[:], 0.0)

    # ---- emit helpers ----
    def proj(name, w_sb, b_sb, o_sb, qp2):
        ps_p = [
            sc_pool.tile([128, 1024], F32, tag="sc", name=f"ps_{name}{a}")
            for a in range(A)
        ]
        for ke in range(KE):
            xt = xs_pool.tile([128, 1024], BF16, tag="xq", name="xt")
            nc.sync.dma_start(xt[:], d[name][qp2, ke])
            for a in range(A):
                for half in range(2):
                    nc.tensor.matmul(
                        ps_p[a][:, 512 * half : 512 * (half + 1)],
                        w_sb[:, ke, 128 * a : 128 * (a + 1)],
                        xt[:, 512 * half : 512 * (half + 1)],
                        start=(ke == 0),
                        stop=(ke == KE - 1),
                    )
        for a in range(A):
            nc.scalar.add(
                o_sb[:, a, 1024 * qp2 : 1024 * (qp2 + 1)],
                ps_p[a][:],
                b_sb[:, a : a + 1],
            )

    def make_vproj_half(kt, half):
        # vT: v in transposed layout: vT[kpos, c] = sum_e v[e,kpos] WvT[e,c]
        def go():
            ps_v = pr_pool.tile([128, 512], F32, tag="pr", name="ps_v")
            for ke in range(KE):
                nc.tensor.matmul(
                    ps_v[:, 0:CH],
                    xv_sb[:, ke, 256 * kt + 128 * half : 256 * kt + 128 * (half + 1)],
                    wv_sb[:, ke, :],
                    start=(ke == 0),
                    stop=(ke == KE - 1),
                )
            kc = 2 * kt + half
            nc.vector.tensor_copy(
                vT_sb[:, kc, :].rearrange("p (h c) -> p h c", h=HPC)[:, :, 0:DH],
                ps_v[:, 0:CH].rearrange("p (h c) -> p h c", h=HPC),
            )
        return go

    def make_qproj1_chunks(a, half):
        # q-projection for q columns 1024+512*half, channels 128a..128a+128,
        # split into two deferred units of 4 ke-steps each (PE burst <1us)
        cell = {}

        def go1():
            ps = pr_pool.tile([128, 512], F32, tag="pr", name="ps_q1")
            cell["ps"] = ps
            for ke in range(KE // 2):
                nc.tensor.matmul(
                    ps[:],
                    wq_sb[:, ke, 128 * a : 128 * (a + 1)],
                    xq1_sb[:, ke, 512 * half : 512 * (half + 1)],
                    start=(ke == 0),
                    stop=False,
                )

        def go2():
            ps = cell["ps"]
            for ke in range(KE // 2, KE):
                nc.tensor.matmul(
                    ps[:],
                    wq_sb[:, ke, 128 * a : 128 * (a + 1)],
                    xq1_sb[:, ke, 512 * half : 512 * (half + 1)],
                    start=False,
                    stop=(ke == KE - 1),
                )
            nc.vector.tensor_scalar_add(
                qp_sb[:, a, 1024 + 512 * half : 1024 + 512 * (half + 1)],
                ps[:],
                bq_sb[:, a : a + 1],
            )
        return [go1, go2]

    # Deferred-emission queue: keeps PE fed with ready work while the
    # scores->exp->mul chain of recent tiles is still in flight.
    DELAY = 8
    deferred = []

    def push(fn):
        deferred.append(fn)
        if len(deferred) > DELAY:
            deferred.pop(0)()

    def flush():
        while deferred:
            deferred.pop(0)()

    def make_outproj(oc, qb):
        def go():
            ps_o = pr_pool.tile([128, 512], F32, tag="pr", name="ps_o")
            for a in range(A):
                nc.tensor.matmul(
                    ps_o[:],
                    wo_sb[:, a, 128 * oc : 128 * (oc + 1)],
                    attn_sb[:, a, 512 * qb : 512 * (qb + 1)],
                    start=(a == 0),
                    stop=(a == A - 1),
                )
            ot = out_pool.tile([128, 512], F32, name="ot")
            nc.vector.tensor_copy(ot[:], ps_o[:])
            nc.sync.dma_start(d["out"][oc, qb], ot[:])
        return go

    def make_attnv(attn_ps, a, u, wt):
        def go():
            for j in range(2):
                h = 2 * a + j
                nc.tensor.matmul(
                    attn_ps[j][:],
                    vT_sb[:, u, 128 * h : 128 * (h + 1)],
                    wt[:, 512 * j : 512 * (j + 1)],
                    start=(u == 0),
                    stop=(u == U - 1),
                )
        return go

    def make_norm(attn_ps_j, a, j, qb):
        def go():
            # the den copy runs on ACT: it becomes ready exactly in the
            # a-boundary exp bubble. (reciprocal_approx_fast reads garbage
            # from PSUM, so den goes through SBUF.)
            den = sm_pool.tile([64, 512], F32, tag="den", name="den")
            if j == 0:
                nc.scalar.copy(den[:], attn_ps_j[64:128, :])
            else:
                nc.vector.tensor_copy(den[:], attn_ps_j[64:128, :])
            rec = sm_pool.tile([64, 512], F32, tag="rec", name="rec")
            nc.vector.reciprocal_approx_fast(rec[:], den[:])
            nc.vector.tensor_mul(
                attn_sb[64 * j : 64 * (j + 1), a, 512 * qb : 512 * (qb + 1)],
                attn_ps_j[0:DH, :],
                rec[:],
            )
        return go

    def attention_qb(qb, extra=()):
        extra = list(extra)
        em2 = []
        for t in range(KT):
            e = em_pool.tile([128, 1024], BF16, tag="em", name=f"em{t}")
            nc.sync.dma_start(e[:], d["emask"][t, qb])
            em2.append(e)
        for a in range(A):
            attn_ps = [
                at_pool.tile([128, 512], F32, tag="at", name=f"attn_ps{j}")
                for j in range(2)
            ] if blevel >= 3 else [None, None]
            for u in range(U):
                t, half = u // 2, u % 2
                ps_s = sc_pool.tile([128, 1024], F32, tag="sc", name="ps_s")
                for j in range(2):
                    nc.tensor.matmul(
                        ps_s[:, 512 * j : 512 * (j + 1)],
                        kp_sb[64 * j : 64 * (j + 1), a, 128 * u : 128 * (u + 1)],
                        qp_sb[64 * j : 64 * (j + 1), a, 512 * qb : 512 * (qb + 1)],
                        start=True,
                        stop=True,
                    )
                if blevel >= 1:
                    et = w_pool.tile([128, 1024], BF16, tag="et", bufs=4)
                    nc.scalar.activation(et[:], ps_s[:], Exp)
                if blevel >= 2:
                    wt = w_pool.tile([128, 1024], BF16, tag="wt", bufs=9)
                    ems = em2[t][:, 512 * half : 512 * (half + 1)]
                    nc.vector.tensor_mul(
                        wt[:].rearrange("p (j c) -> p j c", j=2),
                        et[:].rearrange("p (j c) -> p j c", j=2),
                        ems.unsqueeze(1).broadcast_to((128, 2, 512)),
                    )
                if extra and u % 2 == 1:
                    push(extra.pop(0))
                if blevel >= 3:
                    push(make_attnv(attn_ps, a, u, wt))
            if blevel >= 4:
                push(make_norm(attn_ps[0], a, 0, qb))
                push(make_norm(attn_ps[1], a, 1, qb))
        for fn in extra:
            push(fn)

    # ---- emission schedule ----
    has_a = "A" in phases
    has_b = "B" in phases
    has_c = "C" in phases

    if has_a:
        proj("xk", wk_sb, bk_sb, kp_sb, 0)
        proj("xk", wk_sb, bk_sb, kp_sb, 1)
        proj("xq", wq_sb, bq_sb, qp_sb, 0)
        if not has_b:
            for a in range(A):
                for half in range(2):
                    for fn in make_qproj1_chunks(a, half):
                        fn()
            for kt in range(KT):
                for half in range(2):
                    make_vproj_half(kt, half)()
        else:
            # vproj kt 0..3 in the prefix; the rest interleaves into qb0
            for kt in range(KT // 2):
                for half in range(2):
                    make_vproj_half(kt, half)()
    if has_b:
        for qb in range(QB):
            extra = []
            if has_a and qb == 0:
                extra += [
                    make_vproj_half(kt, half)
                    for kt in range(KT // 2, KT) for half in range(2)
                ]
            if has_a and qb == 1:
                for a in range(A):
                    for half in range(2):
                        extra += make_qproj1_chunks(a, half)
            if has_c and qb >= 1:
                extra += [make_outproj(oc, qb - 1) for oc in range(OC)]
            attention_qb(qb, extra)
        flush()
        if has_c:
            for oc in range(OC):
                make_outproj(oc, QB - 1)()
    elif has_c:
        for oc in range(OC):
            for qb in range(QB):
                make_outproj(oc, qb)()


def build(repeat: int = 1, phases: str = "ABC", blevel: int = 4):
    nc = bacc.Bacc(
        "TRN2",
        target_bir_lowering=False,
        debug=False,
        enable_asserts=False,
        num_devices=NCORES,
    )
    d = {
        # x inputs packed: xq/xk as [qp2, ke, p, 1024]
        "xq": nc.dram_tensor("xq", (QP2, KE, 128, 1024), BF16, kind="ExternalInput").ap(),
        "xk": nc.dram_tensor("xk", (QP2, KE, 128, 1024), BF16, kind="ExternalInput").ap(),
        # xv packed: [p, ke, s]
        "xv": nc.dram_tensor("xv", (128, KE, S), BF16, kind="ExternalInput").ap(),
        # emask packed: [kt, qb, p, 1024] where 1024 = (two, 512)
        "emask": nc.dram_tensor("emask", (KT, QB, 128, 1024), BF16, kind="ExternalInput").ap(),
        # weights packed: [p, ke, c] / [p, a, o]
        "wqT": nc.dram_tensor("wqT", (128, KE, CH), BF16, kind="ExternalInput").ap(),
        "wkT": nc.dram_tensor("wkT", (128, KE, CH), BF16, kind="ExternalInput").ap(),
        "wvT": nc.dram_tensor("wvT", (128, KE, CH), BF16, kind="ExternalInput").ap(),
        "woT": nc.dram_tensor("woT", (128, A, E), BF16, kind="ExternalInput").ap(),
        "bq": nc.dram_tensor("bq", (128, A), F32, kind="ExternalInput").ap(),
        "bk": nc.dram_tensor("bk", (128, A), F32, kind="ExternalInput").ap(),
        # out packed: [oc, qb, p, 512]
        "out": nc.dram_tensor("out", (OC, QB, 128, 512), F32, kind="ExternalOutput").ap(),
    }
    with tile.TileContext(nc) as tc, ExitStack() as ctx:
        tc._emit_ctx = ctx
        if repeat == 1:
            _emit(tc, nc, d, phases, blevel)
        else:
            with tc.For_i(0, repeat, 1):
                _emit(tc, nc, d, phases, blevel)
    nc.compile()
    return nc


def _pack_x(x):  # (1024, S) f32 -> [qp2, ke, p, 1024] bf16
    bf = ml_dtypes.bfloat16
    a = x.reshape(KE, 128, QP2, 1024).transpose(2, 0, 1, 3)
    return np.ascontiguousarray(a).astype(bf)


def _pack_xv(x):  # (1024, S) f32 -> [p, ke, s] bf16
    bf = ml_dtypes.bfloat16
    a = x.reshape(KE, 128, S).transpose(1, 0, 2)
    return np.ascontiguousarray(a).astype(bf)


def _pack_em(em):  # (S, S) f32 (already exp'd) -> [kt, qb, p, (two 512)] bf16
    bf = ml_dtypes.bfloat16
    a = em.reshape(KT, 2, 128, QB, 512).transpose(0, 3, 2, 1, 4).reshape(KT, QB, 128, 1024)
    return np.ascontiguousarray(a).astype(bf)


def _pack_w(wT):  # (E, CH) -> [p, ke, c] bf16
    bf = ml_dtypes.bfloat16
    a = wT.reshape(KE, 128, CH).transpose(1, 0, 2)
    return np.ascontiguousarray(a).astype(bf)


def _pack_wo(woT):  # (CH, E) -> [p, a, o] bf16
    bf = ml_dtypes.bfloat16
    a = woT.reshape(A, 128, E).transpose(1, 0, 2)
    return np.ascontiguousarray(a).astype(bf)


def _pack_b(b):  # (CH,) -> (128, A) f32
    return np.ascontiguousarray(b.reshape(A, 128).T).astype(np.float32)


def prep_inputs(q, k, v, qk_mask, Wq, bq, Wk, bk, Wv, bv, Wo, bo):
    scale = float(DH) ** -0.5
    q2 = np.asarray(q, np.float32).reshape(B, E, S)
    k2 = np.asarray(k, np.float32).reshape(B, E, S)
    v2 = np.asarray(v, np.float32).reshape(B, E, S)
    em = np.exp(np.asarray(qk_mask, np.float32).reshape(B, S, S))
    Wq = np.asarray(Wq, np.float32)
    Wk = np.asarray(Wk, np.float32)
    Wv = np.asarray(Wv, np.float32)
    Wo = np.asarray(Wo, np.float32)
    bqv = np.asarray(bq, np.float32)
    bkv = np.asarray(bk, np.float32)
    bvv = np.asarray(bv, np.float32)
    bov = np.asarray(bo, np.float32)
    host_bias = (Wo @ bvv + bov).astype(np.float32)

    xq = [_pack_x(q2[b]) for b in range(B)]
    xk = [_pack_x(k2[b]) for b in range(B)]
    xv = [_pack_xv(v2[b]) for b in range(B)]
    emp = [_pack_em(em[b]) for b in range(B)]

    in_maps = []
    for c in range(NCORES):
        b, g = divmod(c, GROUPS)
        ch = slice(CH * g, CH * (g + 1))
        in_maps.append(
            {
                "xq": xq[b],
                "xk": xk[b],
                "xv": xv[b],
                "emask": emp[b],
                "wqT": _pack_w(np.ascontiguousarray((scale * Wq[ch]).T)),
                "wkT": _pack_w(np.ascontiguousarray(Wk[ch].T)),
                "wvT": _pack_w(np.ascontiguousarray(Wv[ch].T)),
                "woT": _pack_wo(np.ascontiguousarray(Wo[:, ch].T)),
                "bq": _pack_b(scale * bqv[ch]),
                "bk": _pack_b(bkv[ch]),
            }
        )
    return in_maps, host_bias


def unpack_out(packed):  # [oc, qb, p, 512] -> (E, S)
    return np.ascontiguousarray(
        packed.transpose(0, 2, 1, 3).reshape(E, S)
    )


_NC_CACHE = {}


def kernel(**inputs) -> np.ndarray:
    rep = int(os.environ.get("MHA_REPEAT", "1"))
    if rep not in _NC_CACHE:
        _NC_CACHE[rep] = build(rep)
    nc = _NC_CACHE[rep]
    in_maps, host_bias = prep_inputs(**inputs)
    res = bass_utils.run_bass_kernel_spmd(nc, in_maps, core_ids=list(range(NCORES)))
    out = np.zeros((B, E, 1, S), np.float32)
    for c in range(NCORES):
        b = c // GROUPS
        out[b, :, 0, :] += unpack_out(res.results[c]["out"])
    out += host_bias[None, :, None, None]
    return out
